# revision 54
# baseline (speedup 1.0000x reference)
"""Contrastive loss (SimCLR-style, masked-diagonal logsumexp) on 8 Trainium2
NeuronCores via Bass/Tile.

Math (matches the jax reference):
    a = anchor / ||anchor||_row ; p = positive / ||positive||_row
    F = concat([a, p])                         # [R=2B, D]
    sim = (F F^T) / T with diagonal masked
    lse_i = log(sum_{j!=i} exp(sim_ij))
    pos_i = <a_i, p_i> / T  (duplicated for both halves)
    loss = sum_i (lse_i - pos_i) * lab_i / max(sum_i lab_i, 1)

exp(sim) is symmetric, so only the upper triangle of the 16x16 grid of
512x512 blocks is computed (136 of 256 blocks): each block (I, J) yields
row-sum partials for chunk I (ACT Exp accum) and, when I != J, column-sum
partials for chunk J (ones-vector PE matmul over the exp tile). The two
diagonal blocks are themselves computed as upper triangles at 128-row
band granularity (ragged widths 512/384/256/128); each row's lower part
is recovered by per-band colsums of the symmetric exp tile.

Distribution (uniform SPMD stream, zero collectives): core c owns chunk
rows c and c+8. Step (h, g) computes block (I, (I+g) mod 16) with
I = c + 8h, g = 0..8 for h=0 and g = 0..7 for h=1 — a circular-gap
schedule that covers every unordered block pair exactly once and gives
every core the identical 17-step instruction shape. All per-core
variation lives in the host-side chunk roll (slot s holds global chunk
(c+s) mod 16), so the stationary operands sit at fixed slots 0 and 8.

Features are L2-normalized on the host, scaled by 16 and quantized to
fp8e4 (e4m3); matmuls run in DoubleRow perf mode (2 k-subtiles per
instruction). sim = G * (1/T)/256 rides the ACT Exp scale. The diagonal
of the two diag blocks is pushed to ~-14 in sim units by a DVE
mask-subtract (exp -> ~6e-7, negligible like the reference's
exp(-1e9) = 0); the mask is a windowed [128, 1024] bf16 tensor sliced
per 128-row band. Masking on DVE also frees the PSUM accumulation bank
without a round trip through the Scalar queue.

Off-diag steps are emitted in pairs sharing one 2-bank PSUM tile so a
single ACT Exp covers [128, 1024]; its accumulator then holds the SUM of
the pair's row-sums, which is all the host needs (it only ever sums
row-sum partials per phase). Exp tiles are written back as fp8 so each
column-sum is 2 DoubleRow matmuls against a ones vector (row sums come
from the ACT accumulator, which sums in f32 pre-cast; single-step
groups use a DVE reduce instead, keeping READ_ACCUM off the Scalar
queue). Group order puts the h=1 diag block LAST (no colsum -> minimal
tail: STT -> Exp+accum -> 16 B/partition DMA).

Performance notes (HW-measured): fp8 DoubleRow streams 1 col/cycle at
2.4 GHz (157 TF/s) — the 302 512-col matmuls are a ~65 us floor per
core. The PE DVFS ramp needs ~3 us of CONTINUOUS busy to reach 2.4 GHz
and resets on ~1 us idle gaps, so warmup matmuls lead the stream and
fill the early DMA-wait holes. Slot DMAs issue in consumption order in
landing-gated waves (a probe read after each wave) — otherwise every
transfer shares the DMA round-robin and the first-needed slot lands
~4 us late. Redundant LDWEIGHTS (~135 ns each, not modeled by CoreSim)
are stripped post-finalize when consecutive matmuls share a stationary.

The device ships raw per-group row/column-sum partials; row-sum partials
accumulate in SBUF and leave in a split DMA (groups 0..8 early, group 9
on the tail). The host un-rolls them, adds across cores, and finishes:
loss = sum(lab*(ln(rowsum) - pos))/num_pos.
"""

import os
import numpy as np
import ml_dtypes


# ---------------------------------------------------------------- config ----
class CFG:
    B = 4096
    D = 1024
    NC = 8           # cores
    JW = 512         # chunk width (one PSUM bank of f32)
    JC = 16          # number of row/col chunks (R / JW)
    KC = 8           # k-subtiles of 128
    TEMP = 0.07
    SCALE = 16.0     # fp8 pre-scale; G = SCALE^2 * cos
    MASKSUB = 512.0  # subtracted from G on the diagonal (~2x diag value)
    WARMUP = 40      # PE p-state warmup matmuls ([128, 128] bf16 each)

    @property
    def R(self):
        return 2 * self.B

    # step table: t -> (h, g, slot, isdiag); slots are per-core rolled.
    @property
    def steps(self):
        out = []
        for t in range(9):
            out.append((0, t, t, t == 0))
        for t in range(8):
            out.append((1, t, 8 + t, t == 0))
        return out

    # emission groups: (stationary slot, step list); off-diag steps ride in
    # pairs sharing one 2-bank PSUM tile / ACT instruction. h0 leads with its
    # diag (only needs slot 0 -> earliest start); h1 ends with its diag
    # (no colsum -> minimal tail).
    @property
    def groups(self):
        return [
            (0, [0]), (0, [1, 2]), (0, [3, 4]), (0, [5, 6]), (0, [7, 8]),
            (8, [16]), (8, [10, 11]), (8, [12, 13]), (8, [14, 15]), (8, [9]),
        ]


_BUILD_CACHE = {}


# ----------------------------------------------------------------- build ----
def build_nc(cfg: CFG):
    import concourse.bass as bass  # noqa: F401  (AP helpers live here)
    import concourse.tile as tile
    from contextlib import ExitStack
    from concourse import bacc, mybir

    f32 = mybir.dt.float32
    bf16 = mybir.dt.bfloat16
    f8 = mybir.dt.float8e4
    Act = mybir.ActivationFunctionType
    Alu = mybir.AluOpType

    JW, JC, KC = cfg.JW, cfg.JC, cfg.KC
    K2 = KC // 2                           # DoubleRow: 2 k-subtiles per MM
    perf = mybir.MatmulPerfMode.DoubleRow
    sc = (1.0 / cfg.TEMP) / (cfg.SCALE * cfg.SCALE)
    steps = cfg.steps
    groups = cfg.groups
    nsteps = len(steps)
    ngroups = len(groups)

    nc = bacc.Bacc("TRN2", target_bir_lowering=False, debug=False,
                   num_devices=cfg.NC)

    # partition-major, exactly the SBUF tile layout: multi-slot DMAs iterate
    # src and dst APs in lockstep, so the axis orders must match
    fch = nc.dram_tensor("fchunks", [128, JC, KC, JW], f8,
                         kind="ExternalInput").ap()
    # windowed diag mask: maskd[p, c] = 1 iff c == p + 512; slice
    # [512-128*mt : 1024-128*mt] puts the 1 at column mt*128+p
    maskd = nc.dram_tensor("maskd", [128, 2 * JW], bf16,
                           kind="ExternalInput").ap()
    # cs_out tail: 2 x 768 f32 of diag-block lower-triangle colsum partials
    # (widths 384/256/128 per band, see emit_colsums)
    LOWW = 768
    rs_out = nc.dram_tensor("rs_out", [128, ngroups, 4], f32,
                            kind="ExternalOutput").ap()
    cs_out = nc.dram_tensor("cs_out", [1, nsteps * JW + 2 * LOWW], f32,
                            kind="ExternalOutput").ap()

    with tile.TileContext(nc) as tc, ExitStack() as ctx:
        const = ctx.enter_context(tc.tile_pool(name="const", bufs=1))
        esp2 = ctx.enter_context(tc.tile_pool(name="es2", bufs=4))
        esp1 = ctx.enter_context(tc.tile_pool(name="es1", bufs=2))
        yp = ctx.enter_context(tc.tile_pool(name="y", bufs=4))
        smal = ctx.enter_context(tc.tile_pool(name="small", bufs=1))
        gp = ctx.enter_context(tc.tile_pool(name="g", bufs=3, space="PSUM"))
        csp = ctx.enter_context(tc.tile_pool(name="cs", bufs=2, space="PSUM"))

        # ---- staging -------------------------------------------------------
        fc_sb = const.tile([128, JC, KC, JW], f8)

        # Slot DMAs ride the gpsimd queue in gated waves: after each wave a
        # 4-byte probe read of the wave's tail forces the queue to wait for
        # the landing before issuing the next wave. Without this every
        # transfer is in flight at once and the DMA engines' round-robin
        # starves the first-needed slots (slot 0 was landing ~4 us late).
        # sync carries only slot 1 + the mask; outputs stay off both queues.
        gate = smal.tile([1, 4], f8)

        def wave(slots, probe):
            for dst, src in slots:
                nc.gpsimd.dma_start(dst, src)
            nc.gpsimd.tensor_copy(gate[:], probe)

        nc.sync.dma_start(fc_sb[:, 0, 0:2, :], fch[:, 0, 0:2, :])
        nc.gpsimd.dma_start(fc_sb[:, 0, 2:4, :], fch[:, 0, 2:4, :])
        nc.sync.dma_start(fc_sb[:, 0, 4:6, :], fch[:, 0, 4:6, :])
        nc.gpsimd.dma_start(fc_sb[:, 0, 6:8, :], fch[:, 0, 6:8, :])
        nc.sync.dma_start(fc_sb[:, 1], fch[:, 1])
        maskW = smal.tile([128, 2 * JW], bf16)
        nc.sync.dma_start(maskW[:], maskd)
        wave([(fc_sb[:, 2], fch[:, 2])], fc_sb[0:1, 2, 7, JW - 4:JW])
        wave([(fc_sb[:, 3], fch[:, 3]), (fc_sb[:, 4], fch[:, 4])],
             fc_sb[0:1, 4, 7, JW - 4:JW])
        wave([(fc_sb[:, 5], fch[:, 5]), (fc_sb[:, 6], fch[:, 6])],
             fc_sb[0:1, 6, 7, JW - 4:JW])
        wave([(fc_sb[:, 7], fch[:, 7]), (fc_sb[:, 8:10], fch[:, 8:10])],
             fc_sb[0:1, 9, 7, JW - 4:JW])
        wave([(fc_sb[:, 14:16], fch[:, 14:16]),
              (fc_sb[:, 10:12], fch[:, 10:12])],
             fc_sb[0:1, 11, 7, JW - 4:JW])
        nc.gpsimd.dma_start(fc_sb[:, 12:14], fch[:, 12:14])

        # PE p-state warmup fodder (scratch first: warmup gates on it)
        scratch = smal.tile([128, 128], bf16)
        nc.vector.memset(scratch[:], 1.0)
        ones_bf = smal.tile([128, 1], bf16)
        nc.vector.memset(ones_bf[:], 1.0)
        # warm the Exp ACT table while DMAs stream
        dummy = smal.tile([1, 1], f32)
        nc.vector.memset(dummy[:], 0.0)
        nc.scalar.activation(dummy[:], dummy[:], Act.Exp)
        # [128, 2, 16]: the k-pair step must be even and 16B-aligned for
        # DoubleRow ldweights (s3_lw_dual_fp8_restrictions)
        ones_f8 = smal.tile([128, 2, 16], f8)
        nc.vector.memset(ones_f8[:], 1.0)

        # PE p-state warmup: back-to-back matmuls keep the clock ramping
        # while the first slot DMA lands (reuses a colsum PSUM buffer).
        # wfill() is also sprinkled between the first groups' rounds so the
        # PE never idles (and never drops out of the DVFS ramp) while the
        # early slot DMAs trickle in.
        wps = csp.tile([1, JW], f32, tag="cs")

        def wfill(n):
            for _ in range(n):
                nc.tensor.matmul(wps[:, 0:128], ones_bf[:, 0:1], scratch[:],
                                 start=True, stop=True)

        wfill(cfg.WARMUP)

        rs_all = const.tile([128, ngroups, 4], f32)
        cs_sb = const.tile([1, nsteps * JW + 2 * LOWW], f32)

        # ---- main loop -----------------------------------------------------
        pend = None  # (M, es_g) of previous group, colsums deferred

        def emit_colsums(M, es_g):
            live = [(i, t) for i, t in enumerate(M) if not steps[t][3]]
            if not live:
                # diag block: only the upper-triangle bands were computed
                # (ragged widths). Recover each row's lower part from the
                # symmetric exp tile: colsum band b's rows over its
                # beyond-the-band columns -> rows of bands b+1.. .
                t = M[0]
                base = nsteps * JW + (LOWW if t == 9 else 0)
                off = 0
                for b in range(3):
                    w = JW - 128 * b
                    cp = csp.tile([1, JW], f32, tag="cs")
                    nc.tensor.matmul(cp[:, 0:w - 128], ones_f8[:, 0, 0:1],
                                     es_g[:, b, 0, 128:w],
                                     start=True, stop=True)
                    nc.vector.tensor_copy(
                        cs_sb[:, base + off:base + off + w - 128],
                        cp[:, 0:w - 128])
                    off += w - 128
                nc.sync.dma_start(cs_out[:, base:base + LOWW],
                                  cs_sb[:, base:base + LOWW])
                return
            for i, t in live:
                cp = csp.tile([1, JW], f32, tag="cs")
                for mtp in (0, 2):
                    nc.tensor.matmul(cp[:], ones_f8[:, :, 0:1],
                                     es_g[:, mtp:mtp + 2, i, :],
                                     start=(mtp == 0), stop=(mtp == 2),
                                     perf_mode=perf)
                nc.vector.tensor_copy(cs_sb[:, t * JW:(t + 1) * JW], cp[:])
            t0, t1 = M[0], M[-1] + 1
            nc.sync.dma_start(cs_out[:, t0 * JW:t1 * JW],
                              cs_sb[:, t0 * JW:t1 * JW])

        def single_exp(es_g, src, gidx, mt, accum=False, w=JW):
            # singles: Exp without accum_out (no READ_ACCUM on the Scalar
            # queue -> PSUM frees faster); row sums via a DVE reduce of the
            # fp8 exp tile instead. The LAST group keeps accum_out: its
            # row-sum gates the final rs DMA, and ACT+READ beats
            # ACT -> DVE-reduce on that critical tail path.
            if accum:
                nc.scalar.activation(es_g[:, mt, 0, 0:w], src, Act.Exp,
                                     scale=float(sc),
                                     accum_out=rs_all[:, gidx, mt:mt + 1])
            else:
                nc.scalar.activation(es_g[:, mt, 0, 0:w], src, Act.Exp,
                                     scale=float(sc))
                nc.vector.tensor_reduce(rs_all[:, gidx, mt:mt + 1],
                                        es_g[:, mt, 0, 0:w],
                                        axis=mybir.AxisListType.X,
                                        op=Alu.add)

        def masked_exp(es_g, Gt, b, gidx, mt, accum=False):
            # subtract MASKSUB on the diagonal before Exp (STT on DVE also
            # frees the PSUM bank without waiting on the Scalar queue).
            # Diag tiles are ragged: band mt covers columns mt*128..511, so
            # the self-similarity element sits at LOCAL column p for every
            # band and one mask window fits all.
            w = JW - 128 * mt
            y = yp.tile([128, JW], f32, tag="y")
            nc.vector.scalar_tensor_tensor(
                out=y[:, 0:w], in0=maskW[:, JW:JW + w],
                scalar=-float(cfg.MASKSUB), in1=Gt[:, b, 0:w],
                op0=Alu.mult, op1=Alu.add)
            single_exp(es_g, y[:, 0:w], gidx, mt, accum=accum, w=w)

        for gidx, (S, M) in enumerate(groups):
            nM = len(M)
            isdiag = steps[M[0]][3]
            esp = esp2 if nM == 2 else esp1
            es_g = esp.tile([128, 4, nM, JW], f8, tag="es", name="es")
            if isdiag and gidx == 0:
                # k2-outer: consume k-pairs in DMA-arrival order so the PE
                # starts as soon as the first quarter of slot 0 lands
                GtAB = [gp.tile([128, 2, JW], f32, tag="g", name=f"gtd{h}")
                        for h in range(2)]
                for k2 in range(K2):
                    if k2:
                        wfill(5)  # keep PE busy while the next k-pair lands
                    ksl = slice(2 * k2, 2 * k2 + 2)
                    for mt in range(4):
                        w = JW - 128 * mt
                        nc.tensor.matmul(
                            GtAB[mt // 2][:, mt % 2, 0:w],
                            fc_sb[:, S, ksl, mt * 128:(mt + 1) * 128],
                            fc_sb[:, S, ksl, mt * 128:JW],
                            start=(k2 == 0), stop=(k2 == K2 - 1),
                            perf_mode=perf)
                wfill(5)  # cover the slot-1/2 landing before group 1
                for mt in range(4):
                    masked_exp(es_g, GtAB[mt // 2], mt % 2, gidx, mt)
            else:
                for mt in range(4):
                    Gt = gp.tile([128, 2, JW], f32, tag="g")
                    wd = JW - 128 * mt if isdiag else JW
                    for k2 in range(K2):
                        ksl = slice(2 * k2, 2 * k2 + 2)
                        for i, t in enumerate(M):
                            sl = steps[t][2]
                            nc.tensor.matmul(
                                Gt[:, i, 0:wd],
                                fc_sb[:, S, ksl, mt * 128:(mt + 1) * 128],
                                fc_sb[:, sl, ksl, JW - wd:JW],
                                start=(k2 == 0), stop=(k2 == K2 - 1),
                                perf_mode=perf)
                    if nM == 2:
                        # one Exp over both banks; accum = sum of the pair's
                        # row-sums, which is all the host needs
                        nc.scalar.activation(
                            es_g[:, mt, :, :], Gt[:], Act.Exp,
                            scale=float(sc),
                            accum_out=rs_all[:, gidx, mt:mt + 1])
                    elif isdiag:
                        masked_exp(es_g, Gt, 0, gidx, mt,
                                   accum=(gidx == ngroups - 1))
                    else:
                        single_exp(es_g, Gt[:, 0, :], gidx, mt)
            if pend is not None:
                emit_colsums(*pend)
            pend = (M, es_g)
            if gidx == ngroups - 2:
                # ship groups 0..n-2 row sums early; only the last group's
                # 16B/partition slice rides the tail
                nc.sync.dma_start(rs_out[:, 0:ngroups - 1, :],
                                  rs_all[:, 0:ngroups - 1, :])
        emit_colsums(*pend)
        nc.sync.dma_start(rs_out[:, ngroups - 1:, :],
                          rs_all[:, ngroups - 1:, :])

    nc.finalize()
    if not int(os.environ.get("KERNEL_NO_DEDUP", "0")):
        dedup_ldweights(nc)
    return nc


def dedup_ldweights(nc):
    """Remove redundant InstLdweights from the finalized module.

    tile_legalize splits every matmul into InstLdweights + InstMatmult
    (ldweights=False). When consecutive PE matmuls share the same stationary
    (pair steps; colsum ones-vector chains; warmup), the repeated loads are
    pure overhead (~135 ns each on HW). Drop an InstLdweights when the PE
    array already holds identical weights, moving any waits/updates onto the
    paired matmul when slots allow (TRN2: at most 1 wait / 1 update per
    engine instruction).
    """
    from concourse import mybir

    def ldw_key(ins):
        ap = ins.ins[0]
        return (str(ap), str(ins.perf_mode), str(ins.is_transpose),
                str(ins.tile_position), str(ins.tile_size))

    for b in nc.main_func.blocks:
        insts = list(b.instructions)
        loaded = None
        keep = [True] * len(insts)
        for idx, ins in enumerate(insts):
            if isinstance(ins, mybir.InstLdweights):
                key = ldw_key(ins)
                si = ins.sync_info
                waits = list(si.on_wait) if si else []
                upds = list(si.on_update) if si else []
                if key == loaded:
                    nxt = None
                    for j in range(idx + 1, len(insts)):
                        if isinstance(insts[j],
                                      (mybir.InstMatmult,
                                       mybir.InstLdweights)):
                            nxt = insts[j]
                            break
                    if nxt is not None and isinstance(nxt, mybir.InstMatmult):
                        nsi = nxt.sync_info
                        nw = list(nsi.on_wait) if nsi else []
                        nu = list(nsi.on_update) if nsi else []
                        if len(nw) + len(waits) <= 1 and \
                                len(nu) + len(upds) <= 1:
                            if waits or upds:
                                nxt.sync_info = mybir.SyncInfo(
                                    on_wait=nw + waits,
                                    on_update=nu + upds)
                            keep[idx] = False
                            continue
                loaded = key
        if not all(keep):
            new = [i for i, k in zip(insts, keep) if k]
            del b.instructions[:]
            for i in new:
                b.instructions.append(i)
    return nc


# ------------------------------------------------------------ host side -----
def make_in_maps(cfg: CFG, feats_q: np.ndarray):
    JC, JW, KC = cfg.JC, cfg.JW, cfg.KC
    # X[p, j, k, n] = feats_q[j*JW + n, k*128 + p]  (partition-major)
    X = feats_q.reshape(JC, JW, KC, 128).transpose(3, 0, 2, 1)
    maskW = np.zeros((128, 2 * JW), ml_dtypes.bfloat16)
    maskW[np.arange(128), np.arange(128) + JW] = 1.0
    in_maps = []
    for c in range(cfg.NC):
        roll = [(c + s) % JC for s in range(JC)]
        in_maps.append({"fchunks": np.ascontiguousarray(X[:, roll]),
                        "maskd": maskW})
    return in_maps


LAST_RESULTS = None


def kernel(anchor_features, positive_features, labels):
    global LAST_RESULTS
    from concourse.bass_utils import run_bass_kernel_spmd

    cfg = CFG()
    key = (cfg.B, cfg.D, cfg.NC)
    if key not in _BUILD_CACHE:
        _BUILD_CACHE[key] = build_nc(cfg)
    nc = _BUILD_CACHE[key]

    a = np.asarray(anchor_features, dtype=np.float32)
    p = np.asarray(positive_features, dtype=np.float32)
    lab = np.asarray(labels).astype(np.float64)
    an = a / np.linalg.norm(a, axis=1, keepdims=True)
    pn = p / np.linalg.norm(p, axis=1, keepdims=True)
    cross = np.einsum("ij,ij->i", an, pn, dtype=np.float64) / cfg.TEMP
    feats = np.concatenate([an, pn], axis=0) * cfg.SCALE
    feats_q = feats.astype(ml_dtypes.float8_e4m3)

    in_maps = make_in_maps(cfg, feats_q)
    trace = bool(int(os.environ.get("KERNEL_TRACE", "0")))
    res = run_bass_kernel_spmd(nc, in_maps, list(range(cfg.NC)), trace=trace)
    LAST_RESULTS = res

    # un-roll per-core partials into the global row-sum vector
    rowsum = np.zeros(cfg.R, np.float64)
    steps = cfg.steps
    nsteps = len(steps)
    for c in range(cfg.NC):
        rs = np.asarray(res.results[c]["rs_out"], np.float64)  # [128, 10, 4]
        cs_flat = np.asarray(res.results[c]["cs_out"], np.float64).ravel()
        cs = cs_flat[:nsteps * cfg.JW].reshape(nsteps, cfg.JW)
        low = cs_flat[nsteps * cfg.JW:].reshape(2, 768)
        for gidx, (S, M) in enumerate(cfg.groups):
            I = (c + S) % cfg.JC
            # group accum = sum of its steps' row-sum partials for chunk I
            # (diag groups: upper-triangle bands only)
            rowsum[I * cfg.JW:(I + 1) * cfg.JW] += rs[:, gidx, :].T.reshape(-1)
            for t in M:
                h, g, _slot, isdiag = steps[t]
                if isdiag:
                    # lower-triangle parts of the diag block, recovered from
                    # per-band colsums of the symmetric exp tile
                    lo = low[1 if t == 9 else 0]
                    off = 0
                    for b in range(3):
                        w = 384 - 128 * b
                        r0 = I * cfg.JW + 128 * (b + 1)
                        rowsum[r0:r0 + w] += lo[off:off + w]
                        off += w
                else:
                    J = (I + g) % cfg.JC
                    rowsum[J * cfg.JW:(J + 1) * cfg.JW] += cs[t]

    lse = np.log(rowsum)
    pos2 = np.concatenate([cross, cross])
    lab2 = np.concatenate([lab, lab])
    num_pos = lab2.sum()
    loss = (lab2 * (lse - pos2)).sum() / num_pos if num_pos > 0 else 0.0
    return np.float32(loss)


# revision 55
# speedup vs baseline: 1.0049x; 1.0049x over previous
"""Contrastive loss (SimCLR-style, masked-diagonal logsumexp) on 8 Trainium2
NeuronCores via Bass/Tile.

Math (matches the jax reference):
    a = anchor / ||anchor||_row ; p = positive / ||positive||_row
    F = concat([a, p])                         # [R=2B, D]
    sim = (F F^T) / T with diagonal masked
    lse_i = log(sum_{j!=i} exp(sim_ij))
    pos_i = <a_i, p_i> / T  (duplicated for both halves)
    loss = sum_i (lse_i - pos_i) * lab_i / max(sum_i lab_i, 1)

exp(sim) is symmetric, so only the upper triangle of the 16x16 grid of
512x512 blocks is computed (136 of 256 blocks): each block (I, J) yields
row-sum partials for chunk I (ACT Exp accum) and, when I != J, column-sum
partials for chunk J (ones-vector PE matmul over the exp tile). The two
diagonal blocks are themselves computed as upper triangles at 128-row
band granularity (ragged widths 512/384/256/128); each row's lower part
is recovered by per-band colsums of the symmetric exp tile.

Distribution (uniform SPMD stream, zero collectives): core c owns chunk
rows c and c+8. Step (h, g) computes block (I, (I+g) mod 16) with
I = c + 8h, g = 0..8 for h=0 and g = 0..7 for h=1 — a circular-gap
schedule that covers every unordered block pair exactly once and gives
every core the identical 17-step instruction shape. All per-core
variation lives in the host-side chunk roll (slot s holds global chunk
(c+s) mod 16), so the stationary operands sit at fixed slots 0 and 8.

Features are L2-normalized on the host, scaled by 16 and quantized to
fp8e4 (e4m3); matmuls run in DoubleRow perf mode (2 k-subtiles per
instruction). sim = G * (1/T)/256 rides the ACT Exp scale. The diagonal
of the two diag blocks is pushed to ~-14 in sim units by a DVE
mask-subtract (exp -> ~6e-7, negligible like the reference's
exp(-1e9) = 0); the mask is a windowed [128, 1024] bf16 tensor sliced
per 128-row band. Masking on DVE also frees the PSUM accumulation bank
without a round trip through the Scalar queue.

Off-diag steps are emitted in pairs sharing one 2-bank PSUM tile so a
single ACT Exp covers [128, 1024]; its accumulator then holds the SUM of
the pair's row-sums, which is all the host needs (it only ever sums
row-sum partials per phase). Exp tiles are written back as fp8 so each
column-sum is 2 DoubleRow matmuls against a ones vector (row sums come
from the ACT accumulator, which sums in f32 pre-cast; single-step
groups use a DVE reduce instead, keeping READ_ACCUM off the Scalar
queue). Group order puts the h=1 diag block LAST (no colsum -> minimal
tail: STT -> Exp+accum -> 16 B/partition DMA).

Performance notes (HW-measured): fp8 DoubleRow streams 1 col/cycle at
2.4 GHz (157 TF/s) — the 302 512-col matmuls are a ~65 us floor per
core. The PE DVFS ramp needs ~3 us of CONTINUOUS busy to reach 2.4 GHz
and resets on ~1 us idle gaps, so warmup matmuls lead the stream and
fill the early DMA-wait holes. Slot DMAs issue in consumption order in
landing-gated waves (a probe read after each wave) — otherwise every
transfer shares the DMA round-robin and the first-needed slot lands
~4 us late. Redundant LDWEIGHTS (~135 ns each, not modeled by CoreSim)
are stripped post-finalize when consecutive matmuls share a stationary.

The device ships raw per-group row/column-sum partials; row-sum partials
accumulate in SBUF and leave in a split DMA (groups 0..8 early, group 9
on the tail). The host un-rolls them, adds across cores, and finishes:
loss = sum(lab*(ln(rowsum) - pos))/num_pos.
"""

import os
import numpy as np
import ml_dtypes


# ---------------------------------------------------------------- config ----
class CFG:
    B = 4096
    D = 1024
    NC = 8           # cores
    JW = 512         # chunk width (one PSUM bank of f32)
    JC = 16          # number of row/col chunks (R / JW)
    KC = 8           # k-subtiles of 128
    TEMP = 0.07
    SCALE = 16.0     # fp8 pre-scale; G = SCALE^2 * cos
    MASKSUB = 512.0  # subtracted from G on the diagonal (~2x diag value)
    WARMUP = 40      # PE p-state warmup matmuls ([128, 128] bf16 each)

    @property
    def R(self):
        return 2 * self.B

    # step table: t -> (h, g, slot, isdiag); slots are per-core rolled.
    @property
    def steps(self):
        out = []
        for t in range(9):
            out.append((0, t, t, t == 0))
        for t in range(8):
            out.append((1, t, 8 + t, t == 0))
        return out

    # emission groups: (stationary slot, step list); off-diag steps ride in
    # pairs sharing one 2-bank PSUM tile / ACT instruction. h0 leads with its
    # diag (only needs slot 0 -> earliest start); h1 ends with its diag
    # (no colsum -> minimal tail).
    @property
    def groups(self):
        return [
            (0, [0]), (0, [1, 2]), (0, [3, 4]), (0, [5, 6]), (0, [7, 8]),
            (8, [16]), (8, [10, 11]), (8, [12, 13]), (8, [14, 15]), (8, [9]),
        ]


_BUILD_CACHE = {}


# ----------------------------------------------------------------- build ----
def build_nc(cfg: CFG):
    import concourse.bass as bass  # noqa: F401  (AP helpers live here)
    import concourse.tile as tile
    from contextlib import ExitStack
    from concourse import bacc, mybir

    f32 = mybir.dt.float32
    bf16 = mybir.dt.bfloat16
    f8 = mybir.dt.float8e4
    Act = mybir.ActivationFunctionType
    Alu = mybir.AluOpType

    JW, JC, KC = cfg.JW, cfg.JC, cfg.KC
    K2 = KC // 2                           # DoubleRow: 2 k-subtiles per MM
    perf = mybir.MatmulPerfMode.DoubleRow
    sc = (1.0 / cfg.TEMP) / (cfg.SCALE * cfg.SCALE)
    steps = cfg.steps
    groups = cfg.groups
    nsteps = len(steps)
    ngroups = len(groups)

    nc = bacc.Bacc("TRN2", target_bir_lowering=False, debug=False,
                   num_devices=cfg.NC)

    # partition-major, exactly the SBUF tile layout: multi-slot DMAs iterate
    # src and dst APs in lockstep, so the axis orders must match
    fch = nc.dram_tensor("fchunks", [128, JC, KC, JW], f8,
                         kind="ExternalInput").ap()
    # windowed diag mask: maskd[p, c] = 1 iff c == p + 512; slice
    # [512-128*mt : 1024-128*mt] puts the 1 at column mt*128+p
    maskd = nc.dram_tensor("maskd", [128, 2 * JW], bf16,
                           kind="ExternalInput").ap()
    # cs_out tail: 2 x 768 f32 of diag-block lower-triangle colsum partials
    # (widths 384/256/128 per band, see emit_colsums)
    LOWW = 768
    rs_out = nc.dram_tensor("rs_out", [128, ngroups, 4], f32,
                            kind="ExternalOutput").ap()
    cs_out = nc.dram_tensor("cs_out", [1, nsteps * JW + 2 * LOWW], f32,
                            kind="ExternalOutput").ap()

    with tile.TileContext(nc) as tc, ExitStack() as ctx:
        const = ctx.enter_context(tc.tile_pool(name="const", bufs=1))
        esp2 = ctx.enter_context(tc.tile_pool(name="es2", bufs=4))
        esp1 = ctx.enter_context(tc.tile_pool(name="es1", bufs=2))
        yp = ctx.enter_context(tc.tile_pool(name="y", bufs=4))
        smal = ctx.enter_context(tc.tile_pool(name="small", bufs=1))
        gp = ctx.enter_context(tc.tile_pool(name="g", bufs=3, space="PSUM"))
        csp = ctx.enter_context(tc.tile_pool(name="cs", bufs=2, space="PSUM"))

        # ---- staging -------------------------------------------------------
        fc_sb = const.tile([128, JC, KC, JW], f8)

        # Slot DMAs ride the gpsimd queue in gated waves: after each wave a
        # 4-byte probe read of the wave's tail forces the queue to wait for
        # the landing before issuing the next wave. Without this every
        # transfer is in flight at once and the DMA engines' round-robin
        # starves the first-needed slots (slot 0 was landing ~4 us late).
        # sync carries only slot 1 + the mask; outputs stay off both queues.
        gate = smal.tile([1, 4], f8)

        def wave(slots, probe):
            for dst, src in slots:
                nc.gpsimd.dma_start(dst, src)
            nc.gpsimd.tensor_copy(gate[:], probe)

        nc.sync.dma_start(fc_sb[:, 0, 0:2, :], fch[:, 0, 0:2, :])
        nc.gpsimd.dma_start(fc_sb[:, 0, 2:4, :], fch[:, 0, 2:4, :])
        nc.sync.dma_start(fc_sb[:, 0, 4:6, :], fch[:, 0, 4:6, :])
        nc.gpsimd.dma_start(fc_sb[:, 0, 6:8, :], fch[:, 0, 6:8, :])
        nc.sync.dma_start(fc_sb[:, 1], fch[:, 1])
        maskW = smal.tile([128, 2 * JW], bf16)
        nc.sync.dma_start(maskW[:], maskd)
        wave([(fc_sb[:, 2], fch[:, 2])], fc_sb[0:1, 2, 7, JW - 4:JW])
        wave([(fc_sb[:, 3], fch[:, 3]), (fc_sb[:, 4], fch[:, 4])],
             fc_sb[0:1, 4, 7, JW - 4:JW])
        wave([(fc_sb[:, 5], fch[:, 5]), (fc_sb[:, 6], fch[:, 6])],
             fc_sb[0:1, 6, 7, JW - 4:JW])
        wave([(fc_sb[:, 7], fch[:, 7]), (fc_sb[:, 8:10], fch[:, 8:10])],
             fc_sb[0:1, 9, 7, JW - 4:JW])
        wave([(fc_sb[:, 14:16], fch[:, 14:16]),
              (fc_sb[:, 10:12], fch[:, 10:12])],
             fc_sb[0:1, 11, 7, JW - 4:JW])
        nc.gpsimd.dma_start(fc_sb[:, 12:14], fch[:, 12:14])

        # PE p-state warmup fodder (scratch first: warmup gates on it)
        scratch = smal.tile([128, 128], bf16)
        nc.vector.memset(scratch[:], 1.0)
        ones_bf = smal.tile([128, 1], bf16)
        nc.vector.memset(ones_bf[:], 1.0)
        # warm the Exp ACT table while DMAs stream
        dummy = smal.tile([1, 1], f32)
        nc.vector.memset(dummy[:], 0.0)
        nc.scalar.activation(dummy[:], dummy[:], Act.Exp)
        # [128, 2, 16]: the k-pair step must be even and 16B-aligned for
        # DoubleRow ldweights (s3_lw_dual_fp8_restrictions)
        ones_f8 = smal.tile([128, 2, 16], f8)
        nc.vector.memset(ones_f8[:], 1.0)

        # PE p-state warmup: back-to-back matmuls keep the clock ramping
        # while the first slot DMA lands (reuses a colsum PSUM buffer).
        # wfill() is also sprinkled between the first groups' rounds so the
        # PE never idles (and never drops out of the DVFS ramp) while the
        # early slot DMAs trickle in.
        wps = csp.tile([1, JW], f32, tag="cs")

        def wfill(n):
            for _ in range(n):
                nc.tensor.matmul(wps[:, 0:128], ones_bf[:, 0:1], scratch[:],
                                 start=True, stop=True)

        wfill(cfg.WARMUP)

        rs_all = const.tile([128, ngroups, 4], f32)
        cs_sb = const.tile([1, nsteps * JW + 2 * LOWW], f32)

        # ---- main loop -----------------------------------------------------
        pend = None  # (M, es_g) of previous group, colsums deferred

        def emit_colsums(M, es_g):
            live = [(i, t) for i, t in enumerate(M) if not steps[t][3]]
            if not live:
                # diag block: only the upper-triangle bands were computed
                # (ragged widths). Recover each row's lower part from the
                # symmetric exp tile: colsum band b's rows over its
                # beyond-the-band columns -> rows of bands b+1.. .
                t = M[0]
                base = nsteps * JW + (LOWW if t == 9 else 0)
                off = 0
                for b in range(3):
                    w = JW - 128 * b
                    cp = csp.tile([1, JW], f32, tag="cs")
                    nc.tensor.matmul(cp[:, 0:w - 128], ones_f8[:, 0, 0:1],
                                     es_g[:, b, 0, 128:w],
                                     start=True, stop=True)
                    nc.vector.tensor_copy(
                        cs_sb[:, base + off:base + off + w - 128],
                        cp[:, 0:w - 128])
                    off += w - 128
                nc.sync.dma_start(cs_out[:, base:base + LOWW],
                                  cs_sb[:, base:base + LOWW])
                return
            for i, t in live:
                cp = csp.tile([1, JW], f32, tag="cs")
                for mtp in (0, 2):
                    nc.tensor.matmul(cp[:], ones_f8[:, :, 0:1],
                                     es_g[:, mtp:mtp + 2, i, :],
                                     start=(mtp == 0), stop=(mtp == 2),
                                     perf_mode=perf)
                nc.vector.tensor_copy(cs_sb[:, t * JW:(t + 1) * JW], cp[:])
            t0, t1 = M[0], M[-1] + 1
            nc.sync.dma_start(cs_out[:, t0 * JW:t1 * JW],
                              cs_sb[:, t0 * JW:t1 * JW])

        def single_exp(es_g, src, gidx, mt, accum=False, w=JW):
            # singles: Exp without accum_out (no READ_ACCUM on the Scalar
            # queue -> PSUM frees faster); row sums via a DVE reduce of the
            # fp8 exp tile instead. The LAST group keeps accum_out: its
            # row-sum gates the final rs DMA, and ACT+READ beats
            # ACT -> DVE-reduce on that critical tail path.
            if accum:
                nc.scalar.activation(es_g[:, mt, 0, 0:w], src, Act.Exp,
                                     scale=float(sc),
                                     accum_out=rs_all[:, gidx, mt:mt + 1])
            else:
                nc.scalar.activation(es_g[:, mt, 0, 0:w], src, Act.Exp,
                                     scale=float(sc))
                nc.vector.tensor_reduce(rs_all[:, gidx, mt:mt + 1],
                                        es_g[:, mt, 0, 0:w],
                                        axis=mybir.AxisListType.X,
                                        op=Alu.add)

        def masked_exp(es_g, Gt, b, gidx, mt, accum=False):
            # subtract MASKSUB on the diagonal before Exp (STT on DVE also
            # frees the PSUM bank without waiting on the Scalar queue).
            # Diag tiles are ragged: band mt covers columns mt*128..511, so
            # the self-similarity element sits at LOCAL column p for every
            # band and one mask window fits all.
            w = JW - 128 * mt
            y = yp.tile([128, JW], f32, tag="y")
            nc.vector.scalar_tensor_tensor(
                out=y[:, 0:w], in0=maskW[:, JW:JW + w],
                scalar=-float(cfg.MASKSUB), in1=Gt[:, b, 0:w],
                op0=Alu.mult, op1=Alu.add)
            single_exp(es_g, y[:, 0:w], gidx, mt, accum=accum, w=w)

        for gidx, (S, M) in enumerate(groups):
            nM = len(M)
            isdiag = steps[M[0]][3]
            esp = esp2 if nM == 2 else esp1
            es_g = esp.tile([128, 4, nM, JW], f8, tag="es", name="es")
            if isdiag and gidx == 0:
                # k2-outer: consume k-pairs in DMA-arrival order so the PE
                # starts as soon as the first quarter of slot 0 lands
                GtAB = [gp.tile([128, 2, JW], f32, tag="g", name=f"gtd{h}")
                        for h in range(2)]
                for k2 in range(K2):
                    if k2:
                        wfill(5)  # keep PE busy while the next k-pair lands
                    ksl = slice(2 * k2, 2 * k2 + 2)
                    for mt in range(4):
                        w = JW - 128 * mt
                        nc.tensor.matmul(
                            GtAB[mt // 2][:, mt % 2, 0:w],
                            fc_sb[:, S, ksl, mt * 128:(mt + 1) * 128],
                            fc_sb[:, S, ksl, mt * 128:JW],
                            start=(k2 == 0), stop=(k2 == K2 - 1),
                            perf_mode=perf)
                wfill(5)  # cover the slot-1/2 landing before group 1
                for mt in range(4):
                    masked_exp(es_g, GtAB[mt // 2], mt % 2, gidx, mt)
            else:
                for mt in range(4):
                    # singles only ever use bank 0: a 1-bank tile doubles the
                    # pool's effective depth, decoupling the final diag
                    # group's matmuls from the Scalar ACT backlog
                    Gt = gp.tile([128, nM, JW], f32, tag="g")
                    wd = JW - 128 * mt if isdiag else JW
                    for k2 in range(K2):
                        ksl = slice(2 * k2, 2 * k2 + 2)
                        for i, t in enumerate(M):
                            sl = steps[t][2]
                            nc.tensor.matmul(
                                Gt[:, i, 0:wd],
                                fc_sb[:, S, ksl, mt * 128:(mt + 1) * 128],
                                fc_sb[:, sl, ksl, JW - wd:JW],
                                start=(k2 == 0), stop=(k2 == K2 - 1),
                                perf_mode=perf)
                    if nM == 2:
                        # one Exp over both banks; accum = sum of the pair's
                        # row-sums, which is all the host needs
                        nc.scalar.activation(
                            es_g[:, mt, :, :], Gt[:], Act.Exp,
                            scale=float(sc),
                            accum_out=rs_all[:, gidx, mt:mt + 1])
                    elif isdiag:
                        masked_exp(es_g, Gt, 0, gidx, mt,
                                   accum=(gidx == ngroups - 1))
                    else:
                        single_exp(es_g, Gt[:, 0, :], gidx, mt)
            if pend is not None:
                emit_colsums(*pend)
            pend = (M, es_g)
            if gidx == ngroups - 2:
                # ship groups 0..n-2 row sums early; only the last group's
                # 16B/partition slice rides the tail
                nc.sync.dma_start(rs_out[:, 0:ngroups - 1, :],
                                  rs_all[:, 0:ngroups - 1, :])
        emit_colsums(*pend)
        nc.sync.dma_start(rs_out[:, ngroups - 1:, :],
                          rs_all[:, ngroups - 1:, :])

    nc.finalize()
    if not int(os.environ.get("KERNEL_NO_DEDUP", "0")):
        dedup_ldweights(nc)
    return nc


def dedup_ldweights(nc):
    """Remove redundant InstLdweights from the finalized module.

    tile_legalize splits every matmul into InstLdweights + InstMatmult
    (ldweights=False). When consecutive PE matmuls share the same stationary
    (pair steps; colsum ones-vector chains; warmup), the repeated loads are
    pure overhead (~135 ns each on HW). Drop an InstLdweights when the PE
    array already holds identical weights, moving any waits/updates onto the
    paired matmul when slots allow (TRN2: at most 1 wait / 1 update per
    engine instruction).
    """
    from concourse import mybir

    def ldw_key(ins):
        ap = ins.ins[0]
        return (str(ap), str(ins.perf_mode), str(ins.is_transpose),
                str(ins.tile_position), str(ins.tile_size))

    for b in nc.main_func.blocks:
        insts = list(b.instructions)
        loaded = None
        keep = [True] * len(insts)
        for idx, ins in enumerate(insts):
            if isinstance(ins, mybir.InstLdweights):
                key = ldw_key(ins)
                si = ins.sync_info
                waits = list(si.on_wait) if si else []
                upds = list(si.on_update) if si else []
                if key == loaded:
                    nxt = None
                    for j in range(idx + 1, len(insts)):
                        if isinstance(insts[j],
                                      (mybir.InstMatmult,
                                       mybir.InstLdweights)):
                            nxt = insts[j]
                            break
                    if nxt is not None and isinstance(nxt, mybir.InstMatmult):
                        nsi = nxt.sync_info
                        nw = list(nsi.on_wait) if nsi else []
                        nu = list(nsi.on_update) if nsi else []
                        if len(nw) + len(waits) <= 1 and \
                                len(nu) + len(upds) <= 1:
                            if waits or upds:
                                nxt.sync_info = mybir.SyncInfo(
                                    on_wait=nw + waits,
                                    on_update=nu + upds)
                            keep[idx] = False
                            continue
                loaded = key
        if not all(keep):
            new = [i for i, k in zip(insts, keep) if k]
            del b.instructions[:]
            for i in new:
                b.instructions.append(i)
    return nc


# ------------------------------------------------------------ host side -----
def make_in_maps(cfg: CFG, feats_q: np.ndarray):
    JC, JW, KC = cfg.JC, cfg.JW, cfg.KC
    # X[p, j, k, n] = feats_q[j*JW + n, k*128 + p]  (partition-major)
    X = feats_q.reshape(JC, JW, KC, 128).transpose(3, 0, 2, 1)
    maskW = np.zeros((128, 2 * JW), ml_dtypes.bfloat16)
    maskW[np.arange(128), np.arange(128) + JW] = 1.0
    in_maps = []
    for c in range(cfg.NC):
        roll = [(c + s) % JC for s in range(JC)]
        in_maps.append({"fchunks": np.ascontiguousarray(X[:, roll]),
                        "maskd": maskW})
    return in_maps


LAST_RESULTS = None


def kernel(anchor_features, positive_features, labels):
    global LAST_RESULTS
    from concourse.bass_utils import run_bass_kernel_spmd

    cfg = CFG()
    key = (cfg.B, cfg.D, cfg.NC)
    if key not in _BUILD_CACHE:
        _BUILD_CACHE[key] = build_nc(cfg)
    nc = _BUILD_CACHE[key]

    a = np.asarray(anchor_features, dtype=np.float32)
    p = np.asarray(positive_features, dtype=np.float32)
    lab = np.asarray(labels).astype(np.float64)
    an = a / np.linalg.norm(a, axis=1, keepdims=True)
    pn = p / np.linalg.norm(p, axis=1, keepdims=True)
    cross = np.einsum("ij,ij->i", an, pn, dtype=np.float64) / cfg.TEMP
    feats = np.concatenate([an, pn], axis=0) * cfg.SCALE
    feats_q = feats.astype(ml_dtypes.float8_e4m3)

    in_maps = make_in_maps(cfg, feats_q)
    trace = bool(int(os.environ.get("KERNEL_TRACE", "0")))
    res = run_bass_kernel_spmd(nc, in_maps, list(range(cfg.NC)), trace=trace)
    LAST_RESULTS = res

    # un-roll per-core partials into the global row-sum vector
    rowsum = np.zeros(cfg.R, np.float64)
    steps = cfg.steps
    nsteps = len(steps)
    for c in range(cfg.NC):
        rs = np.asarray(res.results[c]["rs_out"], np.float64)  # [128, 10, 4]
        cs_flat = np.asarray(res.results[c]["cs_out"], np.float64).ravel()
        cs = cs_flat[:nsteps * cfg.JW].reshape(nsteps, cfg.JW)
        low = cs_flat[nsteps * cfg.JW:].reshape(2, 768)
        for gidx, (S, M) in enumerate(cfg.groups):
            I = (c + S) % cfg.JC
            # group accum = sum of its steps' row-sum partials for chunk I
            # (diag groups: upper-triangle bands only)
            rowsum[I * cfg.JW:(I + 1) * cfg.JW] += rs[:, gidx, :].T.reshape(-1)
            for t in M:
                h, g, _slot, isdiag = steps[t]
                if isdiag:
                    # lower-triangle parts of the diag block, recovered from
                    # per-band colsums of the symmetric exp tile
                    lo = low[1 if t == 9 else 0]
                    off = 0
                    for b in range(3):
                        w = 384 - 128 * b
                        r0 = I * cfg.JW + 128 * (b + 1)
                        rowsum[r0:r0 + w] += lo[off:off + w]
                        off += w
                else:
                    J = (I + g) % cfg.JC
                    rowsum[J * cfg.JW:(J + 1) * cfg.JW] += cs[t]

    lse = np.log(rowsum)
    pos2 = np.concatenate([cross, cross])
    lab2 = np.concatenate([lab, lab])
    num_pos = lab2.sum()
    loss = (lab2 * (lse - pos2)).sum() / num_pos if num_pos > 0 else 0.0
    return np.float32(loss)
